# revision 1
# baseline (speedup 1.0000x reference)
"""Single-head causal attention on 8 trn2 NeuronCores.

Problem: x[16, 2048, 1024] fp32, Wq/Wk/Wv[1024, 64] fp32 ->
         out[16, 2048, 64] = softmax(causal(q k^T / sqrt(64))) v

Sharding: data-parallel over batch B=16 -> 2 batches per core, no
collectives. Each core runs an identical (SPMD) Bass program on its own
x shard.

Per-core dataflow (per batch):
  1. DMA x tiles [128, 1024] in natural layout, PE-transpose into
     x^T blocks [C-chunk=128 part, T free] (matmul contracts over the
     partition dim, so the C-contraction of the projections needs
     channels on partitions).
  2. Projections with weights stationary: [Wq|Wk] packed -> one pass
     gives q^T (partitions 0:64) and k^T (partitions 64:128); k^T is
     then partition-shifted to 0:64 by an SBUF->SBUF DMA. Wv pass gives
     v^T; small PE transposes give v natural [T, 64] with a ones column
     appended (the ones column makes the PV matmul emit the softmax
     denominator for free).
  3. Attention in S^T layout: S^T[Tj part, Ti free] tiles via
     lhsT=k^T chunk, rhs=q^T block; exp on ACT (scale=1/8 folded in,
     no max-subtraction - scores are N(0,1)-scale for this problem);
     causal mask on the diagonal chunks via gpsimd affine_select;
     PV accumulates out^T[65, Ti] in PSUM with lhsT=v_ext.
  4. PE-transpose out^T -> out[Ti, 65], divide by the l column, DMA out.

Matmul operand dtype is a knob per group (f32 = 4 cyc/row, f32r = full
rate fp32 storage with rounded-multiplier, bf16 = full rate). The BIR
verifier requires fp32r operands to be *written* as fp32r, so producer
copies write the matmul dtype.
"""

import sys

sys.path.insert(0, "/opt/trn_rl_repo")

import numpy as np

import concourse.bass as bass  # noqa: F401
import concourse.bacc as bacc
import concourse.mybir as mybir
import concourse.tile as tile
from concourse.masks import make_identity
from concourse.bass_utils import run_bass_kernel_spmd

B, T, C, H = 16, 2048, 1024, 64
NCORES = 8
BPC = B // NCORES  # batches per core
CB = C // 128      # 8 contraction chunks
TT = T // 128      # 16 T tiles of 128
NB = T // 512      # 4 T blocks of 512
F32 = mybir.dt.float32
SCALE = float(H) ** -0.5

DT = {"f32": mybir.dt.float32, "f32r": mybir.dt.float32r,
      "bf16": mybir.dt.bfloat16}


def build_program(dt_proj="f32r", dt_qk="f32r", dt_pv="f32r",
                  interleave=True, reps=1):
    from contextlib import ExitStack, nullcontext

    mdt_proj, mdt_qk, mdt_pv = DT[dt_proj], DT[dt_qk], DT[dt_pv]

    nc = bacc.Bacc("TRN2", target_bir_lowering=False, debug=False,
                   num_devices=NCORES)
    x_d = nc.dram_tensor("x", [BPC, T, C], F32, kind="ExternalInput").ap()
    wq_d = nc.dram_tensor("Wq", [C, H], F32, kind="ExternalInput").ap()
    wk_d = nc.dram_tensor("Wk", [C, H], F32, kind="ExternalInput").ap()
    wv_d = nc.dram_tensor("Wv", [C, H], F32, kind="ExternalInput").ap()
    y_d = nc.dram_tensor("y", [BPC, T, H], F32, kind="ExternalOutput").ap()

    with tile.TileContext(nc) as tc, ExitStack() as ctx:
        singles = ctx.enter_context(tc.tile_pool(name="singles", bufs=1))
        xpool = ctx.enter_context(tc.tile_pool(name="xp", bufs=4))
        xTpool = ctx.enter_context(tc.tile_pool(name="xTp", bufs=2))
        qkpool = ctx.enter_context(tc.tile_pool(name="qkp", bufs=2))
        kTpool = ctx.enter_context(tc.tile_pool(name="kTp", bufs=2))
        vTpool = ctx.enter_context(tc.tile_pool(name="vTp", bufs=2))
        vnpool = ctx.enter_context(tc.tile_pool(name="vnp", bufs=2))
        ptpool = ctx.enter_context(tc.tile_pool(name="ptp", bufs=4))
        oexpool = ctx.enter_context(tc.tile_pool(name="oexp", bufs=2))
        ypool = ctx.enter_context(tc.tile_pool(name="yp", bufs=4))
        smallp = ctx.enter_context(tc.tile_pool(name="smp", bufs=4))
        ps_tr = ctx.enter_context(tc.tile_pool(name="pstr", bufs=2, space="PSUM"))
        ps_mm = ctx.enter_context(tc.tile_pool(name="psmm", bufs=3, space="PSUM"))
        ps_oa = ctx.enter_context(tc.tile_pool(name="psoa", bufs=2, space="PSUM"))

        ident = singles.tile([128, 128], F32)
        make_identity(nc, ident[:, :])
        # fp32 staging for weights, then a rounding copy to the matmul dtype
        wqk_s = singles.tile([128, CB, 128], F32)
        nc.sync.dma_start(out=wqk_s[:, :, 0:64],
                          in_=wq_d.rearrange("(c p) h -> p c h", p=128))
        nc.sync.dma_start(out=wqk_s[:, :, 64:128],
                          in_=wk_d.rearrange("(c p) h -> p c h", p=128))
        wv_s = singles.tile([128, CB, 64], F32)
        nc.sync.dma_start(out=wv_s[:, :, :],
                          in_=wv_d.rearrange("(c p) h -> p c h", p=128))
        if dt_proj == "f32":
            wqk, wv = wqk_s, wv_s
        else:
            wqk = singles.tile([128, CB, 128], mdt_proj)
            wv = singles.tile([128, CB, 64], mdt_proj)
            nc.vector.tensor_copy(wqk[:, :, :], wqk_s[:, :, :])
            nc.vector.tensor_copy(wv[:, :, :], wv_s[:, :, :])
        ones_s = singles.tile([128, 4], F32)
        nc.vector.memset(ones_s[:, :], 1.0)
        if dt_pv == "f32":
            ones_c = ones_s
        else:
            ones_c = singles.tile([128, 4], mdt_pv)
            nc.vector.tensor_copy(ones_c[:, :], ones_s[:, :])

        def phase_ab(b, st):
            """Load + transpose x, projections. Yields once per T block."""
            qkT = qkpool.tile([128, T], mdt_qk, tag="qkT")
            kT = kTpool.tile([64, T], mdt_qk, tag="kT")
            vT = vTpool.tile([64, T], F32, tag="vT")
            vn = vnpool.tile([128, TT, 65], mdt_pv, tag="vn")
            st["qkT"], st["kT"], st["vn"] = qkT, kT, vn
            for blk in range(NB):
                xT = xTpool.tile([128, CB, 512], mdt_proj, tag="xT")
                for t4 in range(4):
                    tt = blk * 4 + t4
                    xt = xpool.tile([128, C], F32, tag="x")
                    nc.sync.dma_start(out=xt[:, :],
                                      in_=x_d[b, tt * 128:(tt + 1) * 128, :])
                    # 4 transposes share one PSUM bank -> one wide copy
                    # (amortizes the per-op DVE overhead, DVE was the
                    # co-bottleneck in the timeline model)
                    for g in range(CB // 4):
                        tp4 = ps_tr.tile([128, 512], F32, tag="tr")
                        for q in range(4):
                            ci = 4 * g + q
                            nc.tensor.matmul(tp4[:, q * 128:(q + 1) * 128],
                                             xt[:, ci * 128:(ci + 1) * 128],
                                             ident[:, :], is_transpose=True)
                        nc.vector.tensor_copy(
                            xT[:, 4 * g:4 * g + 4, t4 * 128:(t4 + 1) * 128],
                            tp4[:, :].rearrange("p (c t) -> p c t", c=4))
                pq = ps_mm.tile([128, 512], F32, tag="mm")
                for ci in range(CB):
                    nc.tensor.matmul(pq[:, :], wqk[:, ci, :], xT[:, ci, :],
                                     start=(ci == 0), stop=(ci == CB - 1))
                nc.vector.tensor_copy(qkT[:, blk * 512:(blk + 1) * 512], pq[:, :])
                pv_ = ps_mm.tile([64, 512], F32, tag="mm")
                for ci in range(CB):
                    nc.tensor.matmul(pv_[:, :], wv[:, ci, :], xT[:, ci, :],
                                     start=(ci == 0), stop=(ci == CB - 1))
                nc.vector.tensor_copy(vT[:, blk * 512:(blk + 1) * 512], pv_[:, :])
                # k^T partition shift 64:128 -> 0:64 for this block
                nc.sync.dma_start(out=kT[:, blk * 512:(blk + 1) * 512],
                                  in_=qkT[64:128, blk * 512:(blk + 1) * 512])
                # v natural [Tj, 64] tiles for this block: 4 transposes
                # share one PSUM bank -> one wide copy
                tpv = ps_tr.tile([128, 256], F32, tag="tr")
                for t4 in range(4):
                    tj = blk * 4 + t4
                    nc.tensor.matmul(tpv[:, t4 * 64:(t4 + 1) * 64],
                                     vT[:, tj * 128:(tj + 1) * 128],
                                     ident[0:64, 0:64], is_transpose=True)
                nc.vector.tensor_copy(
                    vn[:, blk * 4:blk * 4 + 4, 0:64],
                    tpv[:, :].rearrange("p (c h) -> p c h", c=4))
                nc.vector.tensor_copy(vn[:, blk * 4:blk * 4 + 4, 64], ones_c[:, :])
                yield

        def phase_c(b, st):
            """Attention. Yields once per Ti block."""
            qkT, kT, vn = st["qkT"], st["kT"], st["vn"]
            for bi in range(NB):
                oacc = ps_oa.tile([65, 512], F32, tag="oa")
                last = 4 * bi + 3
                for j in range(last + 1):
                    r = j - 4 * bi
                    if r <= 0:
                        w, c0 = 512, 0
                    else:
                        w, c0 = 512 - 128 * r, 128 * r
                    stt = ps_mm.tile([128, w], F32, tag="mm")
                    nc.tensor.matmul(
                        stt[:, :], kT[:, j * 128:(j + 1) * 128],
                        qkT[0:64, bi * 512 + c0:(bi + 1) * 512],
                        start=True, stop=True)
                    pt = ptpool.tile([128, w], mdt_pv, tag="pt")
                    nc.scalar.activation(pt[:, :], stt[:, :],
                                         mybir.ActivationFunctionType.Exp,
                                         scale=SCALE)
                    if r >= 0:
                        # keep where (within-tile free idx) >= partition idx
                        nc.gpsimd.affine_select(
                            out=pt[:, :], in_=pt[:, :],
                            compare_op=mybir.AluOpType.is_ge, fill=0.0,
                            base=0, pattern=[[1, w]], channel_multiplier=-1)
                    nc.tensor.matmul(oacc[:, c0:512], vn[:, j, :], pt[:, :],
                                     start=(j == 0), stop=(j == last))
                oex = oexpool.tile([65, 512], F32, tag="oex")
                nc.vector.tensor_copy(oex[:, :], oacc[:, :])
                for t4 in range(4):
                    ot = ps_tr.tile([128, 65], F32, tag="tr")
                    nc.tensor.matmul(ot[:, :], oex[:, t4 * 128:(t4 + 1) * 128],
                                     ident[0:65, 0:65], is_transpose=True)
                    linv = smallp.tile([128, 1], F32, tag="linv")
                    nc.vector.reciprocal(linv[:, :], ot[:, 64:65])
                    yt = ypool.tile([128, 64], F32, tag="yt")
                    nc.vector.tensor_scalar_mul(yt[:, :], ot[:, 0:64], linv[:, :])
                    row = bi * 512 + t4 * 128
                    nc.sync.dma_start(out=y_d[b, row:row + 128, :], in_=yt[:, :])
                yield

        def drain(g):
            for _ in g:
                pass

        def body():
            states = [dict() for _ in range(BPC)]
            if not interleave or BPC == 1:
                for b in range(BPC):
                    drain(phase_ab(b, states[b]))
                    drain(phase_c(b, states[b]))
            else:
                # AB(0) | then alternate C(0) units with AB(1) units | C(1)
                drain(phase_ab(0, states[0]))
                c0 = phase_c(0, states[0])
                ab1 = phase_ab(1, states[1])
                done_c0 = done_ab1 = False
                while not (done_c0 and done_ab1):
                    if not done_c0:
                        done_c0 = next(c0, "end") == "end"
                    if not done_ab1:
                        done_ab1 = next(ab1, "end") == "end"
                drain(phase_c(1, states[1]))

        if reps == 1:
            body()
        else:
            with tc.For_i(0, reps, 1):
                body()

    nc.compile()
    return nc


_CACHE = {}


def _get_program(**kw):
    key = tuple(sorted(kw.items()))
    if key not in _CACHE:
        _CACHE[key] = build_program(**kw)
    return _CACHE[key]


def run_sharded(x, Wq, Wk, Wv, trace=False, **build_kw):
    """Run on 8 cores, return (y_full, BassKernelResults)."""
    nc = _get_program(**build_kw)
    x = np.ascontiguousarray(np.asarray(x, dtype=np.float32))
    Wq = np.ascontiguousarray(np.asarray(Wq, dtype=np.float32))
    Wk = np.ascontiguousarray(np.asarray(Wk, dtype=np.float32))
    Wv = np.ascontiguousarray(np.asarray(Wv, dtype=np.float32))
    xs = x.reshape(NCORES, BPC, T, C)
    in_maps = [{"x": np.ascontiguousarray(xs[i]), "Wq": Wq, "Wk": Wk, "Wv": Wv}
               for i in range(NCORES)]
    res = run_bass_kernel_spmd(nc, in_maps, list(range(NCORES)), trace=trace)
    y = np.stack([res.results[i]["y"] for i in range(NCORES)], axis=0)
    return y.reshape(B, T, H), res


def kernel(x, Wq, Wk, Wv):
    y, _ = run_sharded(x, Wq, Wk, Wv, trace=False)
    return y


# ---------------- timing support (no NTFF profiler in this container) ----


def make_runner(nc, n_iter=1):
    """Build a reusable sharded jit callable for `nc` (mirrors
    bass2jax.run_bass_via_pjrt's multi-core path, without donation so
    device inputs can be reused across timed calls). n_iter > 1 chains
    the NEFF invocation serially (output buffers fed back as the next
    call's output-operands) so per-invocation time can be measured as a
    slope, independent of the ~90 ms axon dispatch floor."""
    import jax
    from jax.sharding import Mesh, PartitionSpec
    try:
        from jax.experimental.shard_map import shard_map
    except ImportError:  # newer jax
        from jax.shard_map import shard_map
    from concourse import bass2jax
    bass2jax.install_neuronx_cc_hook()

    part_name = (nc.partition_id_tensor.name if nc.partition_id_tensor
                 else None)
    in_names, out_names, out_avals, zero_outs = [], [], [], []
    for alloc in nc.m.functions[0].allocations:
        if not isinstance(alloc, mybir.MemoryLocationSet):
            continue
        name = alloc.memorylocations[0].name
        if alloc.kind == "ExternalInput":
            if name != part_name:
                in_names.append(name)
        elif alloc.kind == "ExternalOutput":
            out_names.append(name)
            shape = tuple(alloc.tensor_shape)
            dtype = mybir.dt.np(alloc.dtype)
            out_avals.append(jax.core.ShapedArray(shape, dtype))
            zero_outs.append(np.zeros(shape, dtype))
    n_params = len(in_names)
    all_names = in_names + out_names
    if part_name is not None:
        all_names = all_names + [part_name]

    def _body(*args):
        ins = list(args[:n_params])
        youts = list(args[n_params:n_params + len(out_names)])
        for _ in range(n_iter):
            operands = ins + youts
            if part_name is not None:
                operands.append(bass2jax.partition_id_tensor())
            outs = bass2jax._bass_exec_p.bind(
                *operands, out_avals=tuple(out_avals),
                in_names=tuple(all_names), out_names=tuple(out_names),
                lowering_input_output_aliases=(),
                sim_require_finite=True, sim_require_nnan=True, nc=nc)
            youts = list(outs)
        return tuple(youts)

    devices = jax.devices()[:NCORES]
    mesh = Mesh(np.asarray(devices), ("core",))
    in_specs = (PartitionSpec("core"),) * (n_params + len(out_names))
    out_specs = (PartitionSpec("core"),) * len(out_names)
    fn = jax.jit(shard_map(_body, mesh=mesh, in_specs=in_specs,
                           out_specs=out_specs, check_rep=False),
                 keep_unused=True)
    return fn, in_names, zero_outs, mesh


def _timed_calls(fn, dev_in, iters):
    import time as _time
    import jax
    out = fn(*dev_in)
    jax.block_until_ready(out)
    ts = []
    for _ in range(iters):
        t0 = _time.perf_counter_ns()
        out = fn(*dev_in)
        jax.block_until_ready(out)
        ts.append(_time.perf_counter_ns() - t0)
    ts.sort()
    return ts


def time_calls(nc, in_maps, iters=10):
    """Sorted wall times (ns) of warm sharded calls of nc's NEFF."""
    import jax
    from jax.sharding import NamedSharding, PartitionSpec
    fn, in_names, zero_outs, mesh = make_runner(nc, n_iter=1)
    sh = NamedSharding(mesh, PartitionSpec("core"))
    concat = [np.concatenate([np.asarray(m[n]) for m in in_maps], axis=0)
              for n in in_names]
    concat += [np.zeros((NCORES * z.shape[0], *z.shape[1:]), z.dtype)
               for z in zero_outs]
    dev_in = [jax.device_put(a, sh) for a in concat]
    return _timed_calls(fn, dev_in, iters)


_BASELINE = {}


def baseline_nc():
    """Tiny kernel to measure the axon dispatch floor."""
    if "nc" in _BASELINE:
        return _BASELINE["nc"]
    nc = bacc.Bacc("TRN2", target_bir_lowering=False, debug=False,
                   num_devices=NCORES)
    a = nc.dram_tensor("a", [128, 128], F32, kind="ExternalInput").ap()
    b = nc.dram_tensor("b", [128, 128], F32, kind="ExternalOutput").ap()
    with tile.TileContext(nc) as tc:
        with tc.tile_pool(name="p", bufs=1) as pool:
            t = pool.tile([128, 128], F32)
            nc.sync.dma_start(out=t[:, :], in_=a)
            nc.sync.dma_start(out=b, in_=t[:, :])
    nc.compile()
    _BASELINE["nc"] = nc
    return nc



# revision 29
# speedup vs baseline: 1.0160x; 1.0160x over previous
"""Single-head causal attention on 8 trn2 NeuronCores.

Problem: x[16, 2048, 1024] fp32, Wq/Wk/Wv[1024, 64] fp32 ->
         out[16, 2048, 64] = softmax(causal(q k^T / sqrt(64))) v

Sharding: data-parallel over batch B=16 -> 2 batches per core, no
collectives. Each core runs an identical (SPMD) Bass program on its own
x shard.

v2 schedule — block-streaming: causality means attention for query
block bi only needs k/v blocks <= bi, so each batch runs as 4 units of
[load+transpose+project 512 rows of x] then [attention for that query
block]. The AB work of unit n+1 (DMA/DVE/PE-transpose heavy) overlaps
the C work of unit n (ACT-exp/PE-matmul heavy) through the per-engine
queues, killing the long attention-only tail of the v1 two-phase
schedule.

Other deltas vs v1:
  - x dram tensor is declared f32r (same bits as f32): PE transposes of
    x run at 1.5 cyc/row instead of fp32's 2.0.
  - weight DMAs (strided, ~8.5us) go on the Activation HWDGE queue so
    the SP queue starts streaming x tiles at t=0.
  - y output DMAs are deferred by one unit so they never head-of-line
    block the x loads on the SP queue.
  - PSUM->SBUF copies are split across DVE/ACT/Pool to keep DVE off the
    critical path.
  - attention S^T matmuls read k^T directly from partitions 64:128 of
    the packed [q|k]^T projection output (no partition-shift DMA).
  - S matmuls issue 2 ahead of the matching PV so PE never waits the
    exp latency.
"""

import sys

sys.path.insert(0, "/opt/trn_rl_repo")

import numpy as np

import concourse.bass as bass  # noqa: F401
import concourse.bacc as bacc
import concourse.mybir as mybir
import concourse.tile as tile
from concourse.masks import make_identity
from concourse.bass_utils import run_bass_kernel_spmd

B, T, C, H = 16, 2048, 1024, 64
NCORES = 8
BPC = B // NCORES  # batches per core
CB = C // 128      # 8 contraction chunks
TT = T // 128      # 16 T tiles of 128
NB = T // 512      # 4 T blocks of 512
F32 = mybir.dt.float32
F32R = mybir.dt.float32r
SCALE = float(H) ** -0.5

DT = {"f32": mybir.dt.float32, "f32r": mybir.dt.float32r,
      "bf16": mybir.dt.bfloat16}


def build_program(dt_qk="f32r", dt_pv="f32r", kt_mode="shift",
                  xcopy="vvavvavv", sahead=2, reps=1):
    # NB: gpsimd cannot access PSUM on HW, so only "v"/"a" are legal in
    # xcopy (all these copies read PSUM).
    from contextlib import ExitStack

    mdt_qk, mdt_pv = DT[dt_qk], DT[dt_pv]

    nc = bacc.Bacc("TRN2", target_bir_lowering=False, debug=False,
                   num_devices=NCORES)
    # x declared f32r: identical bits to f32, but PE transposes of f32r
    # inputs run 1.5 cyc/row vs fp32's 2.0.
    x_d = nc.dram_tensor("x", [BPC, T, C], F32R, kind="ExternalInput").ap()
    wq_d = nc.dram_tensor("Wq", [C, H], F32, kind="ExternalInput").ap()
    wk_d = nc.dram_tensor("Wk", [C, H], F32, kind="ExternalInput").ap()
    wv_d = nc.dram_tensor("Wv", [C, H], F32, kind="ExternalInput").ap()
    y_d = nc.dram_tensor("y", [BPC, T, H], F32, kind="ExternalOutput").ap()

    copy_eng = {"v": None, "a": None, "p": None}  # filled after nc exists

    with tile.TileContext(nc) as tc, ExitStack() as ctx:
        singles = ctx.enter_context(tc.tile_pool(name="singles", bufs=1))
        xpool = ctx.enter_context(tc.tile_pool(name="xp", bufs=8))
        xTpool = ctx.enter_context(tc.tile_pool(name="xTp", bufs=2))
        qkpool = ctx.enter_context(tc.tile_pool(name="qkp", bufs=2))
        vTpool = ctx.enter_context(tc.tile_pool(name="vTp", bufs=2))
        vnpool = ctx.enter_context(tc.tile_pool(name="vnp", bufs=2))
        ptpool = ctx.enter_context(tc.tile_pool(name="ptp", bufs=4))
        oexpool = ctx.enter_context(tc.tile_pool(name="oexp", bufs=2))
        ypool = ctx.enter_context(tc.tile_pool(name="yp", bufs=10))
        smallp = ctx.enter_context(tc.tile_pool(name="smp", bufs=4))
        # PSUM: 8 banks of 2KB. tr/proj/out-transposes share a 4-buf ring;
        # S tiles 3 (sahead=2 keeps 3 alive); out accumulator 1 (its drain
        # is deferred into the next unit, freeing the bank before the next
        # C phase needs it).
        ps_tr = ctx.enter_context(tc.tile_pool(name="pstr", bufs=4, space="PSUM"))
        ps_s = ctx.enter_context(tc.tile_pool(name="pss", bufs=3, space="PSUM"))
        ps_oa = ctx.enter_context(tc.tile_pool(name="psoa", bufs=1, space="PSUM"))

        def cp(which, out, in_):
            eng = copy_eng[which]
            if eng is nc.scalar:
                eng.copy(out, in_)
            else:
                eng.tensor_copy(out, in_)

        copy_eng["v"] = nc.vector
        copy_eng["a"] = nc.scalar
        copy_eng["p"] = nc.gpsimd

        ident_f = singles.tile([128, 128], F32)
        make_identity(nc, ident_f[:, :])
        ident = singles.tile([128, 128], F32R)
        nc.vector.tensor_copy(ident[:, :], ident_f[:, :])
        ones16 = singles.tile([128, TT], F32)
        nc.vector.memset(ones16[:, :], 1.0)

        wq_c = singles.tile([128, 512], F32)
        wk_c = singles.tile([128, 512], F32)
        wv_c = singles.tile([128, 512], F32)
        wqk = singles.tile([128, CB, 128], F32R)
        wv = singles.tile([128, CB, 64], F32R)

        def load_weights():
            # Weights load CONTIGUOUSLY (128 x 2KB descriptors instead of
            # 3072 strided 256B ones, ~12x fewer DMA-engine ns) on the ACT
            # queue. Contiguous layout means partition p holds rows
            # 8p..8p+7, so the contraction is chunked by (row mod 8):
            # chunk r = rows {8p+r}. The x^T transposes select the matching
            # strided column subsets, so q/k/v come out identical (the
            # C-sum is just reordered).
            nc.scalar.dma_start(out=wq_c[:, :],
                                in_=wq_d.rearrange("(p r) h -> p (r h)", p=128))
            nc.scalar.dma_start(out=wk_c[:, :],
                                in_=wk_d.rearrange("(p r) h -> p (r h)", p=128))
            nc.scalar.dma_start(out=wv_c[:, :],
                                in_=wv_d.rearrange("(p r) h -> p (r h)", p=128))
            nc.vector.tensor_copy(wqk[:, :, 0:64],
                                  wq_c[:, :].rearrange("p (r h) -> p r h", r=CB))
            nc.vector.tensor_copy(wqk[:, :, 64:128],
                                  wk_c[:, :].rearrange("p (r h) -> p r h", r=CB))
            nc.vector.tensor_copy(wv[:, :, :],
                                  wv_c[:, :].rearrange("p (r h) -> p r h", r=CB))

        def body():
            pending_y = []      # deferred (ytile, b, row) output DMAs
            pending_drain = []  # deferred (oacc, b, bi, state) drain work
            prefetched = {}     # (b, tt) -> xt tile with DMA already issued

            def fetch_x(b, tt):
                xt = xpool.tile([128, C], F32R, tag="x")
                nc.sync.dma_start(out=xt[:, :],
                                  in_=x_d[b, tt * 128:(tt + 1) * 128, :])
                return xt

            def flush_y():
                for yt, b, row in pending_y:
                    nc.sync.dma_start(out=y_d[b, row:row + 128, :], in_=yt[:, :])
                pending_y.clear()

            def drain_piece(t4):
                # One query-tile step of a deferred unit drain: out^T
                # [65, 512] PSUM -> transposed, normalized y tile. Deferred
                # by one unit and spread across its t4 loop so neither the
                # reciprocal's wait on the attention result nor the
                # ot-ring reuse ever head-of-line blocks a queue.
                if not pending_drain:
                    return
                oacc, b, bi, st = pending_drain[0]
                if "oex" not in st:
                    oex = oexpool.tile([65, 512], F32, tag="oex")
                    nc.vector.tensor_copy(oex[:, :], oacc[:, :])
                    st["oex"] = oex
                oex = st["oex"]
                # fp32 transpose: the fp32r ISA path requires even innermost
                # dst counts, which [128, 65] violates
                ot = ps_s.tile([128, 65], F32, tag="s", name="ot")
                nc.tensor.matmul(ot[:, :], oex[:, t4 * 128:(t4 + 1) * 128],
                                 ident_f[0:65, 0:65], is_transpose=True)
                linv = smallp.tile([128, 1], F32, tag="linv")
                nc.vector.reciprocal(linv[:, :], ot[:, 64:65])
                yt = ypool.tile([128, 64], F32, tag="yt")
                nc.vector.tensor_scalar_mul(yt[:, :], ot[:, 0:64], linv[:, :])
                pending_y.append((yt, b, bi * 512 + t4 * 128))
                if t4 == 3:
                    pending_drain.pop(0)

            # bootstrap: first unit's x tiles ahead of the weight DMAs so
            # the DMA engines deliver x tile 0 first
            for tt in range(4):
                prefetched[(0, tt)] = fetch_x(0, tt)
            load_weights()

            for b in range(BPC):
                qkT = qkpool.tile([128, T], mdt_qk, tag="qkT")
                kTt = None
                if kt_mode == "shift":
                    kTt = qkpool.tile([64, T], mdt_qk, tag="kTs", name="kTt")
                vn = vnpool.tile([128, TT, 65], mdt_pv, tag="vn")
                # ones column for the PV denominator trick (copy, not
                # memset: memset can't write f32r)
                nc.vector.tensor_copy(vn[:, :, 64], ones16[:, :])

                for bi in range(NB):
                    # ---- AB(bi): load x block, transpose, project ----
                    xT = xTpool.tile([128, CB, 512], F32R, tag="xT")
                    ci = 0  # copy-engine cursor
                    for t4 in range(4):
                        tt = bi * 4 + t4
                        xt = prefetched.pop((b, tt), None)
                        if xt is None:
                            xt = fetch_x(b, tt)
                        xtr = xt[:, :].rearrange("p (c r) -> p r c", r=CB)
                        for g in range(CB // 4):
                            tp4 = ps_tr.tile([128, 512], F32R, tag="tr")
                            for q in range(4):
                                # strided column subset {8c + r}: output
                                # partition c matches weight chunk r's rows
                                nc.tensor.matmul(tp4[:, q * 128:(q + 1) * 128],
                                                 xtr[:, 4 * g + q, :],
                                                 ident[:, :], is_transpose=True)
                            cp(xcopy[ci % len(xcopy)],
                               xT[:, 4 * g:4 * g + 4, t4 * 128:(t4 + 1) * 128],
                               tp4[:, :].rearrange("p (c t) -> p c t", c=4))
                            ci += 1
                        drain_piece(t4)  # unit n-1's drain, one step per t4
                    flush_y()     # unit n-2's output DMAs (behind x loads)
                    pq = ps_tr.tile([128, 512], F32, tag="tr")
                    for cc in range(CB):
                        nc.tensor.matmul(pq[:, :], wqk[:, cc, :], xT[:, cc, :],
                                         start=(cc == 0), stop=(cc == CB - 1))
                    nc.vector.tensor_copy(qkT[:, bi * 512:(bi + 1) * 512],
                                          pq[:, :])
                    pv_ = ps_tr.tile([64, 512], F32, tag="tr")
                    for cc in range(CB):
                        nc.tensor.matmul(pv_[:, :], wv[:, cc, :], xT[:, cc, :],
                                         start=(cc == 0), stop=(cc == CB - 1))
                    vT = vTpool.tile([64, 512], F32R, tag="vT")
                    nc.vector.tensor_copy(vT[:, :], pv_[:, :])
                    if kt_mode == "shift":
                        # partition shift 64:128 -> 0:64. Kept on the SP
                        # queue: the ACT queue would let the scheduler order
                        # exps ahead of it (exp waits S waits shift -> 7us
                        # head-of-line stall).
                        nc.sync.dma_start(
                            out=kTt[:, bi * 512:(bi + 1) * 512],
                            in_=qkT[64:128, bi * 512:(bi + 1) * 512])
                    # v natural tiles for this block
                    tpv = ps_tr.tile([128, 256], F32R, tag="tr")
                    for t4 in range(4):
                        nc.tensor.matmul(tpv[:, t4 * 64:(t4 + 1) * 64],
                                         vT[:, t4 * 128:(t4 + 1) * 128],
                                         ident[0:64, 0:64], is_transpose=True)
                    nc.vector.tensor_copy(
                        vn[:, bi * 4:bi * 4 + 4, 0:64],
                        tpv[:, :].rearrange("p (c h) -> p c h", c=4))

                    # ---- C(bi): attention for query block bi ----
                    last = 4 * bi + 3
                    kT = qkT[64:128, :] if kt_mode == "offset" else kTt[:, :]

                    def s_tile(j):
                        r = j - 4 * bi
                        if r <= 0:
                            w, c0 = 512, 0
                        else:
                            w, c0 = 512 - 128 * r, 128 * r
                        stt = ps_s.tile([128, w], F32, tag="s")
                        nc.tensor.matmul(
                            stt[:, :], kT[:, j * 128:(j + 1) * 128],
                            qkT[0:64, bi * 512 + c0:(bi + 1) * 512],
                            start=True, stop=True)
                        pt = ptpool.tile([128, w], mdt_pv, tag="pt")
                        nc.scalar.activation(pt[:, :], stt[:, :],
                                             mybir.ActivationFunctionType.Exp,
                                             scale=SCALE)
                        if r >= 0:
                            # keep where (within-tile free idx) >= partition idx
                            nc.gpsimd.affine_select(
                                out=pt[:, :], in_=pt[:, :],
                                compare_op=mybir.AluOpType.is_ge, fill=0.0,
                                base=0, pattern=[[1, w]], channel_multiplier=-1)
                        return pt, c0

                    oacc = ps_oa.tile([65, 512], F32, tag="oa")
                    pts = {}
                    for j in range(min(sahead, last + 1)):
                        pts[j] = s_tile(j)
                    for j in range(last + 1):
                        if j + sahead <= last:
                            pts[j + sahead] = s_tile(j + sahead)
                        pt, c0 = pts.pop(j)
                        nc.tensor.matmul(oacc[:, c0:512], vn[:, j, :], pt[:, :],
                                         start=(j == 0), stop=(j == last))

                    # drain deferred into the next unit's AB section (the
                    # final unit drains right away — nothing follows)
                    pending_drain.append((oacc, b, bi, {}))
            while pending_drain:
                for t4 in range(4):
                    drain_piece(t4)
            flush_y()

        if reps == 1:
            body()
        else:
            with tc.For_i(0, reps, 1):
                body()

    nc.compile()
    return nc


_CACHE = {}


def _get_program(**kw):
    key = tuple(sorted(kw.items()))
    if key not in _CACHE:
        _CACHE[key] = build_program(**kw)
    return _CACHE[key]


def run_sharded(x, Wq, Wk, Wv, trace=False, **build_kw):
    """Run on 8 cores, return (y_full, BassKernelResults)."""
    nc = _get_program(**build_kw)
    x = np.ascontiguousarray(np.asarray(x, dtype=np.float32))
    Wq = np.ascontiguousarray(np.asarray(Wq, dtype=np.float32))
    Wk = np.ascontiguousarray(np.asarray(Wk, dtype=np.float32))
    Wv = np.ascontiguousarray(np.asarray(Wv, dtype=np.float32))
    xs = x.reshape(NCORES, BPC, T, C)
    in_maps = [{"x": np.ascontiguousarray(xs[i]), "Wq": Wq, "Wk": Wk, "Wv": Wv}
               for i in range(NCORES)]
    res = run_bass_kernel_spmd(nc, in_maps, list(range(NCORES)), trace=trace)
    y = np.stack([res.results[i]["y"] for i in range(NCORES)], axis=0)
    return y.reshape(B, T, H), res


def kernel(x, Wq, Wk, Wv):
    y, _ = run_sharded(x, Wq, Wk, Wv, trace=False)
    return y


# ---------------- timing support (no NTFF profiler in this container) ----


def make_runner(nc, n_iter=1):
    """Build a reusable sharded jit callable for `nc` (mirrors
    bass2jax.run_bass_via_pjrt's multi-core path, without donation so
    device inputs can be reused across timed calls). n_iter > 1 chains
    the NEFF invocation serially (output buffers fed back as the next
    call's output-operands) so per-invocation time can be measured as a
    slope, independent of the ~90 ms axon dispatch floor."""
    import jax
    from jax.sharding import Mesh, PartitionSpec
    try:
        from jax.experimental.shard_map import shard_map
    except ImportError:  # newer jax
        from jax.shard_map import shard_map
    from concourse import bass2jax
    bass2jax.install_neuronx_cc_hook()

    part_name = (nc.partition_id_tensor.name if nc.partition_id_tensor
                 else None)
    in_names, out_names, out_avals, zero_outs = [], [], [], []
    for alloc in nc.m.functions[0].allocations:
        if not isinstance(alloc, mybir.MemoryLocationSet):
            continue
        name = alloc.memorylocations[0].name
        if alloc.kind == "ExternalInput":
            if name != part_name:
                in_names.append(name)
        elif alloc.kind == "ExternalOutput":
            out_names.append(name)
            shape = tuple(alloc.tensor_shape)
            dtype = mybir.dt.np(alloc.dtype)
            out_avals.append(jax.core.ShapedArray(shape, dtype))
            zero_outs.append(np.zeros(shape, dtype))
    n_params = len(in_names)
    all_names = in_names + out_names
    if part_name is not None:
        all_names = all_names + [part_name]

    def _body(*args):
        ins = list(args[:n_params])
        youts = list(args[n_params:n_params + len(out_names)])
        for _ in range(n_iter):
            operands = ins + youts
            if part_name is not None:
                operands.append(bass2jax.partition_id_tensor())
            outs = bass2jax._bass_exec_p.bind(
                *operands, out_avals=tuple(out_avals),
                in_names=tuple(all_names), out_names=tuple(out_names),
                lowering_input_output_aliases=(),
                sim_require_finite=True, sim_require_nnan=True, nc=nc)
            youts = list(outs)
        return tuple(youts)

    devices = jax.devices()[:NCORES]
    mesh = Mesh(np.asarray(devices), ("core",))
    in_specs = (PartitionSpec("core"),) * (n_params + len(out_names))
    out_specs = (PartitionSpec("core"),) * len(out_names)
    fn = jax.jit(shard_map(_body, mesh=mesh, in_specs=in_specs,
                           out_specs=out_specs, check_rep=False),
                 keep_unused=True)
    return fn, in_names, zero_outs, mesh


def _timed_calls(fn, dev_in, iters):
    import time as _time
    import jax
    out = fn(*dev_in)
    jax.block_until_ready(out)
    ts = []
    for _ in range(iters):
        t0 = _time.perf_counter_ns()
        out = fn(*dev_in)
        jax.block_until_ready(out)
        ts.append(_time.perf_counter_ns() - t0)
    ts.sort()
    return ts


def time_calls(nc, in_maps, iters=10):
    """Sorted wall times (ns) of warm sharded calls of nc's NEFF."""
    import jax
    from jax.sharding import NamedSharding, PartitionSpec
    fn, in_names, zero_outs, mesh = make_runner(nc, n_iter=1)
    sh = NamedSharding(mesh, PartitionSpec("core"))
    concat = [np.concatenate([np.asarray(m[n]) for m in in_maps], axis=0)
              for n in in_names]
    concat += [np.zeros((NCORES * z.shape[0], *z.shape[1:]), z.dtype)
               for z in zero_outs]
    dev_in = [jax.device_put(a, sh) for a in concat]
    return _timed_calls(fn, dev_in, iters)


_BASELINE = {}


def baseline_nc():
    """Tiny kernel to measure the axon dispatch floor."""
    if "nc" in _BASELINE:
        return _BASELINE["nc"]
    nc = bacc.Bacc("TRN2", target_bir_lowering=False, debug=False,
                   num_devices=NCORES)
    a = nc.dram_tensor("a", [128, 128], F32, kind="ExternalInput").ap()
    b = nc.dram_tensor("b", [128, 128], F32, kind="ExternalOutput").ap()
    with tile.TileContext(nc) as tc:
        with tc.tile_pool(name="p", bufs=1) as pool:
            t = pool.tile([128, 128], F32)
            nc.sync.dma_start(out=t[:, :], in_=a)
            nc.sync.dma_start(out=b, in_=t[:, :])
    nc.compile()
    _BASELINE["nc"] = nc
    return nc
